# revision 32
# baseline (speedup 1.0000x reference)
"""MiTA sparse attention kernel for Trainium2 (8 NeuronCores, Bass/Tile).

Sharding: data-parallel over batch B=16 -> 2 batches per core; all 12 heads
of a batch are processed on the same core.

Math (per batch b, head h; d=64, M=25 experts, kv_topk=12, router_topk=2):
  qkv = x @ Wqkv ; router = AdaptiveAvgPool(q-grid)
  rak = router k^T ; kidx = top12(rak) ; gate = q router^T ; top2 experts/query
  single softmax over {agent logits (25)} U {selected experts' top12 keys}
  out = (e_a @ (softmax(rak*s) @ v) + e_m @ v[kidx]) / denom ; proj.

Implementation notes:
  - selection chain (qk^T, rak, gate, router) in fp32: lower precision flips
    top-k selections (tf32 measured 2.7e-1 rel err; bf16 2.8e-1).
  - value path bf16 (rel-max error ~4e-3 with zero flips).
  - router = pool(q) = pool(x) @ Wq: pooling commutes with the linear map, so
    pool x^T with 2-stage DVE window reduces (independent of the heavy qk^T
    matmuls) and get router^T via a small exact fp32 matmul.
  - moba branch: full-577-key attention weighted by the multiplicity mask
    W[n,j] = sum_m sel[n,m]*mask12[m,j] in {0,1,2} (exact 0/1 matmul in bf16)
    in transposed space so every contraction is matmul-native.
  - softmax runs unstabilized (logit scale ~0.3) = max-subtracted reference.
  - denominators come from ones-augmented value matrices; the divide runs on
    the Pool engine (gpsimd) with a partition broadcast, no DVE involvement.
  - phase order: k-tiles -> v -> rak/top-12 -> PT/av -> q-tiles -> gate/sel
    -> e_a^T -> EW loop -> proj, so the top-k/PT/av chains (DVE/ACT) overlap
    the PE-bound fp32 qk^T matmul window.
  - element-wise work is spread over DVE / ACT / Pool(gpsimd) for occupancy;
    heads' outputs are paired [128, N] so the projection contracts 128 rows.
"""

import sys

for _p in ("/opt/trn_rl_repo",):
    if _p not in sys.path:
        sys.path.insert(0, _p)

from contextlib import ExitStack

import numpy as np
import ml_dtypes

import concourse.bacc as bacc
import concourse.tile as tile
import concourse.mybir as mybir
from concourse.bass_utils import run_bass_kernel_spmd
from concourse.masks import make_identity

FP32 = mybir.dt.float32
BF16 = mybir.dt.bfloat16
ALU = mybir.AluOpType
ACTF = mybir.ActivationFunctionType
AX = mybir.AxisListType

B, N, C = 16, 577, 768
H, D, M, POOL = 12, 64, 25, 5
NB = 2  # batches per core
NCORES = 8
SCALE = float(D) ** -0.5  # 0.125
NEGBIG = -1e30
NTS = [(i * 128, min(128, N - i * 128)) for i in range((N + 127) // 128)]  # 5
CTS = 6  # 128-col tiles per 768
import os
PHASES = int(os.environ.get("MITA_PHASES", "9"))

# adaptive-pool 1D bins of the 24-token grid axis: (start, len)
_BINS = [(int(np.floor(i * 24 / POOL)),
          int(np.ceil((i + 1) * 24 / POOL)) - int(np.floor(i * 24 / POOL)))
         for i in range(POOL)]
# weight 1/(ny*nx) for region m = r*5 + c
_WPOOL = [1.0 / (_BINS[m // POOL][1] * _BINS[m % POOL][1]) for m in range(M)]


def _emit(tc, io):
    nc = tc.nc
    ctx = tc._ctx

    p_const = ctx.enter_context(tc.tile_pool(name="const", bufs=1))
    p_w = ctx.enter_context(tc.tile_pool(name="work", bufs=1))
    p_ew = ctx.enter_context(tc.tile_pool(name="ew", bufs=8))
    p_out = ctx.enter_context(tc.tile_pool(name="pout", bufs=1))
    # PSUM pools: single-bank tiles; 8 banks total (3 + 3 + 2).
    ps_a = ctx.enter_context(tc.tile_pool(name="ps_a", bufs=3, space="PSUM"))
    ps_w = ctx.enter_context(tc.tile_pool(name="ps_w", bufs=3, space="PSUM"))
    ps_v = ctx.enter_context(tc.tile_pool(name="ps_v", bufs=2, space="PSUM"))

    # ---- constants / weights ----
    ident_bf = p_const.tile([128, 128], BF16, tag="idbf")
    make_identity(nc, ident_bf[:])
    ones_bf = p_const.tile([1, 128], BF16, tag="ones")
    nc.vector.memset(ones_bf[:], 1.0)
    wpool = p_const.tile([128, M], FP32, tag="wpool")
    for m in range(M):
        nc.vector.memset(wpool[:, m:m + 1], _WPOOL[m])

    # weight DMAs are interleaved with the first batch's x loads below so
    # the first qk^T matmul isn't stuck behind ~7MB of weight traffic
    wqk_sb, wv_sb, wproj_sb = [], [], []
    bproj_sb = None

    for b in range(NB):
        # ---- load x^T (fp32 + bf16), interleaved with weights on b=0 ----
        xT32 = []
        for kc in range(CTS):
            if b == 0:
                w = p_const.tile([128, 2 * C], FP32, tag=f"wqk{kc}",
                                 name=f"wqk{kc}")
                nc.sync.dma_start(w[:], io["wqk"][kc * 128:(kc + 1) * 128, :])
                wqk_sb.append(w)
            t = p_w.tile([128, N], FP32, tag=f"w{kc}", name=f"xT32_{kc}")
            nc.sync.dma_start(t[:], io["xT_f32"][b, kc * 128:(kc + 1) * 128, :])
            xT32.append(t)
        xTbf = []
        for kc in range(CTS):
            t = p_w.tile([128, N], BF16, tag=f"t{kc}", name=f"xTbf_{kc}")
            nc.sync.dma_start(t[:], io["xT_bf16"][b, kc * 128:(kc + 1) * 128, :])
            xTbf.append(t)
        if b == 0:
            for kc in range(CTS):
                w = p_const.tile([128, C], BF16, tag=f"wv{kc}", name=f"wv{kc}")
                nc.sync.dma_start(w[:], io["wv"][kc * 128:(kc + 1) * 128, :])
                wv_sb.append(w)
            for hp in range(H // 2):
                w = p_const.tile([128, C], BF16, tag=f"wp{hp}", name=f"wp{hp}")
                nc.sync.dma_start(w[:], io["wproj"][hp * 128:(hp + 1) * 128, :])
                wproj_sb.append(w)
            bproj_sb = p_const.tile([1, C], BF16, tag="bproj")
            nc.sync.dma_start(bproj_sb[:], io["bproj"][:, :])

        # ---- xpool: 2-stage windowed sums over the 24x24 token grid ----
        xpool = []
        for kc in range(CTS):
            grid = xT32[kc][:, 0:576].rearrange("p (y x) -> p y x", x=24)
            tmp = p_w.tile([128, POOL * 24], FP32, tag="xptmp", bufs=2)
            tv = tmp[:].rearrange("p (w y) -> p w y", y=24)
            for wi, (x0, nx) in enumerate(_BINS):
                nc.vector.tensor_reduce(tv[:, wi, :].unsqueeze(-1),
                                        grid[:, :, x0:x0 + nx],
                                        axis=AX.X, op=ALU.add)
            xp = p_w.tile([128, 32], FP32, tag=f"xp{kc}", name=f"xp{kc}")
            for vi, (y0, ny) in enumerate(_BINS):
                nc.vector.tensor_reduce(
                    xp[:, vi * POOL:(vi + 1) * POOL]
                    .rearrange("p (w o) -> p w o", o=1),
                    tv[:, :, y0:y0 + ny], axis=AX.X, op=ALU.add)
            nc.gpsimd.tensor_mul(xp[:, 0:M], xp[:, 0:M], wpool[:])
            nc.gpsimd.memset(xp[:, M:32], 0.0)
            xpool.append(xp)

        # ---- router^T via exact fp32 matmul: [128c, 32] per ct ----
        routerT32, routerTbf = [], []
        for ct in range(CTS):
            pr = ps_v.tile([128, 32], FP32, tag="bank_v")
            for kc in range(CTS):
                nc.tensor.matmul(pr[:, :], wqk_sb[kc][:, ct * 128:(ct + 1) * 128],
                                 xpool[kc][:, :], start=(kc == 0), stop=(kc == 5))
            t32 = p_w.tile([128, 32], FP32, tag=f"rt{ct}", name=f"rT32_{ct}")
            nc.scalar.copy(t32[:], pr[:])
            tbf = p_w.tile([128, 32], BF16, tag=f"rtb{ct}", name=f"rTbf_{ct}")
            nc.vector.tensor_copy(tbf[:], t32[:])
            routerT32.append(t32)
            routerTbf.append(tbf)

        # ---- qk^T fp32 (+bf16 copy): k tiles first so the rak/top-12
        # chain overlaps the remaining PE-bound matmuls ----
        qkT32, qkTbf = [None] * (2 * CTS), [None] * (2 * CTS)
        for ct in list(range(CTS, 2 * CTS)) + list(range(CTS)):
            pa = ps_a.tile([128, 512], FP32, tag="bank_a")
            pb = ps_a.tile([128, 65], FP32, tag="bank_a")
            for kc in range(CTS):
                nc.tensor.matmul(pa[:, :], wqk_sb[kc][:, ct * 128:(ct + 1) * 128],
                                 xT32[kc][:, 0:512], start=(kc == 0), stop=(kc == 5))
            for kc in range(CTS):
                nc.tensor.matmul(pb[:, :], wqk_sb[kc][:, ct * 128:(ct + 1) * 128],
                                 xT32[kc][:, 512:577], start=(kc == 0), stop=(kc == 5))
            tag32 = f"q{ct}" if ct < CTS else f"k{ct - CTS}"
            t32 = p_w.tile([128, N], FP32, tag=tag32, name=f"qkT32_{ct}")
            nc.scalar.copy(t32[:, 0:512], pa[:])
            nc.scalar.copy(t32[:, 512:577], pb[:])
            tbf = p_w.tile([128, N], BF16, tag=f"qkbf{ct}", name=f"qkTbf_{ct}")
            nc.gpsimd.tensor_copy(tbf[:, :], t32[:, :])
            qkT32[ct] = t32
            qkTbf[ct] = tbf

        # ---- v natural bf16 with ones-augmentation: [n, 12*65] ----
        v_sb = []
        for i, (n0, nsz) in enumerate(NTS):
            pa = ps_a.tile([128, 512], FP32, tag="bank_a")
            pb = ps_a.tile([128, 256], FP32, tag="bank_a")
            for kc in range(CTS):
                nc.tensor.matmul(pa[:nsz, :], xTbf[kc][:, n0:n0 + nsz],
                                 wv_sb[kc][:, 0:512], start=(kc == 0), stop=(kc == 5))
            for kc in range(CTS):
                nc.tensor.matmul(pb[:nsz, :], xTbf[kc][:, n0:n0 + nsz],
                                 wv_sb[kc][:, 512:768], start=(kc == 0), stop=(kc == 5))
            t = p_w.tile([128, H * 65], BF16, tag=f"v{i}", name=f"v_{i}")
            nc.scalar.copy(
                t[:nsz].rearrange("p (h e) -> p h e", e=65)[:, 0:8, 0:64],
                pa[:nsz].rearrange("p (h e) -> p h e", e=64))
            nc.scalar.copy(
                t[:nsz].rearrange("p (h e) -> p h e", e=65)[:, 8:12, 0:64],
                pb[:nsz].rearrange("p (h e) -> p h e", e=64))
            nc.gpsimd.memset(
                t[:nsz].rearrange("p (h e) -> p h e", e=65)[:, :, 64:65], 1.0)
            v_sb.append(t)

        if PHASES < 2:
            continue
        # ---- rak (fp32) -> top12 threshold -> mask12, early (needs only
        # k tiles + router) ----
        mask12_g = []
        for g2 in range(6):
            rak_sb = p_w.tile([64, N], FP32, tag="rak_sb", name="rak_sb",
                              bufs=2)
            for hh in range(2):
                h = g2 * 2 + hh
                b32 = hh * 32
                rk = (h % 2) * 64
                kt = 6 + h // 2
                ra = ps_a.tile([32, 512], FP32, tag="bank_a", name="ra")
                rb_ = ps_a.tile([32, 65], FP32, tag="bank_a", name="rb_")
                nc.tensor.matmul(ra[:, :],
                                 routerT32[h // 2][rk:rk + 64, :],
                                 qkT32[kt][rk:rk + 64, 0:512],
                                 start=True, stop=True)
                nc.tensor.matmul(rb_[:, :],
                                 routerT32[h // 2][rk:rk + 64, :],
                                 qkT32[kt][rk:rk + 64, 512:577],
                                 start=True, stop=True)
                if hh == 0:
                    nc.scalar.copy(rak_sb[b32:b32 + 32, 0:512], ra[:, :])
                    nc.scalar.copy(rak_sb[b32:b32 + 32, 512:577], rb_[:, :])
                else:
                    nc.vector.tensor_copy(rak_sb[b32:b32 + 32, 0:512], ra[:, :])
                    nc.vector.tensor_copy(rak_sb[b32:b32 + 32, 512:577],
                                          rb_[:, :])
            r8 = p_w.tile([64, 8], FP32, tag="r8", bufs=2)
            rr = p_w.tile([64, N], FP32, tag="rr", name="rr", bufs=1)
            r8b = p_w.tile([64, 8], FP32, tag="r8b", bufs=2)
            nc.vector.max(out=r8[:], in_=rak_sb[:])
            nc.vector.match_replace(out=rr[:], in_to_replace=r8[:],
                                    in_values=rak_sb[:], imm_value=NEGBIG)
            nc.vector.max(out=r8b[:], in_=rr[:])
            mask12 = p_w.tile([64, N], BF16, tag=f"mask12_{g2}",
                              name=f"mask12_{g2}")
            nc.gpsimd.tensor_scalar(mask12[:], rak_sb[:], r8b[:, 3:4], None,
                                    op0=ALU.is_ge)
            mask12_g.append(mask12)

        if PHASES < 3:
            continue
        # ---- PT = exp(SCALE * rak^T) per j-chunk, all heads packed ----
        PT_e, PT_o = [], []
        for i, (j0, jsz) in enumerate(NTS):
            rt_e = ps_a.tile([128, 6 * 32], FP32, tag="bank_a")
            rt_o = ps_a.tile([128, 6 * 32], FP32, tag="bank_a")
            for h in range(H):
                rk = (h % 2) * 64
                dst = rt_o if (h % 2) else rt_e
                nc.tensor.matmul(
                    dst[:jsz, (h // 2) * 32:(h // 2 + 1) * 32],
                    qkTbf[6 + h // 2][rk:rk + 64, j0:j0 + jsz],
                    routerTbf[h // 2][rk:rk + 64, :],
                    start=True, stop=True)
            te = p_w.tile([128, 6 * 32], BF16, tag=f"pte{i}", name=f"PTe{i}")
            to = p_w.tile([128, 6 * 32], BF16, tag=f"pto{i}", name=f"PTo{i}")
            nc.scalar.activation(te[:jsz, :], rt_e[:jsz, :], ACTF.Exp,
                                 scale=SCALE)
            nc.scalar.activation(to[:jsz, :], rt_o[:jsz, :], ACTF.Exp,
                                 scale=SCALE)
            PT_e.append(te)
            PT_o.append(to)

        # ---- agent values av = (PT^T v)/colsum, per-head [32, 65] ----
        av_h = []
        for h in range(H):
            PTx = PT_o if (h % 2) else PT_e
            sg = (h // 2) * 32
            t = p_w.tile([32, 65], BF16, tag=f"av_{h}", name=f"av_{h}")
            nc.vector.memset(t[:], 0.0)
            au = ps_v.tile([32, 65], FP32, tag="bank_v", name="au")
            for i, (j0, jsz) in enumerate(NTS):
                nc.tensor.matmul(
                    au[:, :],
                    PTx[i][:jsz, sg:sg + 32],
                    v_sb[i][:jsz, h * 65:(h + 1) * 65],
                    start=(i == 0), stop=(i == 4))
            rp = p_w.tile([32, 1], FP32, tag="avrec", bufs=2)
            nc.vector.reciprocal(rp[0:M, :], au[0:M, 64:65])
            nc.vector.tensor_scalar(t[0:M, 0:64], au[0:M, 0:64],
                                    rp[0:M, :], None, op0=ALU.mult)
            nc.gpsimd.memset(t[0:M, 64:65], 1.0)
            av_h.append(t)

        if PHASES < 4:
            continue
        # ---- gate (fp32, natural) -> sel; transpose sel to [m, n] ----
        selT = [p_w.tile([64, N], BF16, tag=f"selT{g2}", name=f"selT{g2}")
                for g2 in range(6)]
        for i, (n0, nsz) in enumerate(NTS):
            # mms with different lhsT partition bases must not share a psum
            # bank unfenced (HW crash) -> split by head parity
            gp_e = ps_v.tile([128, 6 * 32], FP32, tag="bank_v")
            gp_o = ps_v.tile([128, 6 * 32], FP32, tag="bank_v")
            for h in range(H):
                rk = (h % 2) * 64
                dst = gp_o if (h % 2) else gp_e
                nc.tensor.matmul(
                    dst[:nsz, (h // 2) * 32:(h // 2 + 1) * 32],
                    qkT32[h // 2][rk:rk + 64, n0:n0 + nsz],
                    routerT32[h // 2][rk:rk + 64, :],
                    start=True, stop=True)
            gate_sb = p_w.tile([128, H * 32], FP32, tag="gate", bufs=2)
            gv = gate_sb[:nsz].rearrange("p (h e) -> p h e", e=32)
            nc.scalar.copy(gv[:, 0:H:2, :],
                           gp_e[:nsz].rearrange("p (h e) -> p h e", e=32))
            nc.scalar.copy(gv[:, 1:H:2, :],
                           gp_o[:nsz].rearrange("p (h e) -> p h e", e=32))
            nc.gpsimd.memset(
                gate_sb[:nsz].rearrange("p (h e) -> p h e", e=32)[:, :, M:32],
                NEGBIG)
            sel_sb = p_w.tile([128, H * 32], BF16, tag="sel", bufs=2)
            m8 = p_w.tile([128, 8], FP32, tag="m8", bufs=2)
            for h in range(H):
                seg = slice(h * 32, (h + 1) * 32)
                nc.vector.max(out=m8[:nsz, :], in_=gate_sb[:nsz, seg])
                nc.gpsimd.tensor_scalar(
                    sel_sb[:nsz, seg], gate_sb[:nsz, seg], m8[:nsz, 1:2], None,
                    op0=ALU.is_ge)
            for ch in range(3):
                pt = ps_w.tile([128, 128], BF16, tag="bank_w")
                nc.tensor.matmul(pt[0:128, 0:nsz],
                                 sel_sb[:nsz, ch * 128:(ch + 1) * 128],
                                 ident_bf[0:nsz, 0:nsz],
                                 is_transpose=True, start=True, stop=True,
                                 skip_group_check=True)
                nc.scalar.copy(selT[2 * ch][:, n0:n0 + nsz], pt[0:64, 0:nsz])
                nc.vector.tensor_copy(selT[2 * ch + 1][:, n0:n0 + nsz],
                                      pt[64:128, 0:nsz])

        # ---- e_a^T direct: exp(SCALE * (router^T)^T q^T) per head ----
        eaT = []
        for h in range(H):
            rk = (h % 2) * 64
            ga = ps_w.tile([32, 512], FP32, tag="bank_w", name="ga")
            gb = ps_w.tile([32, 65], FP32, tag="bank_w", name="gb")
            nc.tensor.matmul(ga[:, :], routerTbf[h // 2][rk:rk + 64, 0:32],
                             qkTbf[h // 2][rk:rk + 64, 0:512],
                             start=True, stop=True)
            nc.tensor.matmul(gb[:, :], routerTbf[h // 2][rk:rk + 64, 0:32],
                             qkTbf[h // 2][rk:rk + 64, 512:577],
                             start=True, stop=True)
            tag = f"q{h}" if h < 6 else f"ea{h - 6}"
            t = p_w.tile([32, N], BF16, tag=tag, name=f"eaT{h}")
            nc.scalar.activation(t[0:32, 0:512], ga[:, :], ACTF.Exp, scale=SCALE)
            nc.scalar.activation(t[0:32, 512:577], gb[:, :], ACTF.Exp,
                                 scale=SCALE)
            eaT.append(t)

        # ---- EW loop per head: W^T, qk^T, exp, mask-multiply, value ----
        outT_pairs = [p_out.tile([128, N], BF16, tag=f"outP{hp}",
                                 name=f"outP{hp}") for hp in range(H // 2)]
        for h in range(H):
            g2 = h // 2
            b32 = (h % 2) * 32
            rk = (h % 2) * 64
            mask12 = mask12_g[g2]
            ew = []
            for i, (j0, jsz) in enumerate(NTS):
                # W^T[j, n] = mask12^T @ sel^T  (exact in bf16)
                wt_a = ps_w.tile([128, 512], FP32, tag="bank_w")
                wt_b = ps_w.tile([128, 65], FP32, tag="bank_w")
                nc.tensor.matmul(wt_a[:jsz, :],
                                 mask12[b32:b32 + 32, j0:j0 + jsz],
                                 selT[g2][b32:b32 + 32, 0:512],
                                 start=True, stop=True)
                nc.tensor.matmul(wt_b[:jsz, :],
                                 mask12[b32:b32 + 32, j0:j0 + jsz],
                                 selT[g2][b32:b32 + 32, 512:577],
                                 start=True, stop=True)
                # qk^T[j, n] (bf16 value path)
                qm_a = ps_a.tile([128, 512], FP32, tag="bank_a")
                qm_b = ps_a.tile([128, 65], FP32, tag="bank_a")
                kt = 6 + h // 2
                qt = h // 2
                nc.tensor.matmul(qm_a[:jsz, :],
                                 qkTbf[kt][rk:rk + 64, j0:j0 + jsz],
                                 qkTbf[qt][rk:rk + 64, 0:512],
                                 start=True, stop=True)
                nc.tensor.matmul(qm_b[:jsz, :],
                                 qkTbf[kt][rk:rk + 64, j0:j0 + jsz],
                                 qkTbf[qt][rk:rk + 64, 512:577],
                                 start=True, stop=True)
                e_a = p_w.tile([128, 512], BF16, tag="exp_a", bufs=6)
                e_b = p_w.tile([128, 65], BF16, tag="exp_b", bufs=6)
                nc.scalar.activation(e_a[:jsz, :], qm_a[:jsz, :], ACTF.Exp,
                                     scale=SCALE)
                nc.scalar.activation(e_b[:jsz, :], qm_b[:jsz, :], ACTF.Exp,
                                     scale=SCALE)
                t = p_ew.tile([128, N], BF16, tag="ew")
                nc.vector.tensor_tensor(t[:jsz, 0:512], e_a[:jsz, :],
                                        wt_a[:jsz, :], op=ALU.mult)
                nc.vector.tensor_tensor(t[:jsz, 512:577], e_b[:jsz, :],
                                        wt_b[:jsz, :], op=ALU.mult)
                ew.append(t)
            # numT [65, 577] = av_aug^T e_a^T + v_aug^T EW^T
            val_a = ps_v.tile([65, 512], FP32, tag="bank_v")
            val_b = ps_v.tile([65, 65], FP32, tag="bank_v")
            nc.tensor.matmul(val_a[:, :], av_h[h][0:32, :],
                             eaT[h][0:32, 0:512],
                             start=True, stop=False)
            nc.tensor.matmul(val_b[:, :], av_h[h][0:32, :],
                             eaT[h][0:32, 512:577],
                             start=True, stop=False)
            for i, (j0, jsz) in enumerate(NTS):
                nc.tensor.matmul(val_a[:, :],
                                 v_sb[i][:jsz, h * 65:(h + 1) * 65],
                                 ew[i][:jsz, 0:512],
                                 start=False, stop=(i == 4))
                nc.tensor.matmul(val_b[:, :],
                                 v_sb[i][:jsz, h * 65:(h + 1) * 65],
                                 ew[i][:jsz, 512:577],
                                 start=False, stop=(i == 4))
            numT = p_w.tile([65, N], FP32, tag="numT", name="numT", bufs=2)
            nc.scalar.copy(numT[:, 0:512], val_a[:])
            nc.scalar.copy(numT[:, 512:577], val_b[:])
            den1 = p_w.tile([1, N], FP32, tag="den1", bufs=1)
            nc.sync.dma_start(den1[0:1, :], numT[64:65, :])
            nc.vector.reciprocal(den1[0:1, :], den1[0:1, :])
            # broadcast the reciprocal and multiply on the Pool engine
            rb = p_w.tile([64, N], FP32, tag="rb", name="rb", bufs=1)
            nc.gpsimd.partition_broadcast(rb[:, :], den1[0:1, :], channels=64)
            dst = outT_pairs[h // 2]
            rows = slice((h % 2) * 64, (h % 2) * 64 + 64)
            nc.gpsimd.tensor_tensor(dst[rows, :], numT[0:64, :], rb[:, :],
                                    op=ALU.mult)

        # ---- proj: out[n, c'] = attnout @ Wproj + bproj, head-paired ----
        if PHASES < 9:
            continue
        for i, (n0, nsz) in enumerate(NTS):
            pr_a = ps_w.tile([128, 512], FP32, tag="bank_w")
            pr_b = ps_w.tile([128, 256], FP32, tag="bank_w")
            for hp in range(H // 2):
                nc.tensor.matmul(pr_a[:nsz, :], outT_pairs[hp][:, n0:n0 + nsz],
                                 wproj_sb[hp][:, 0:512],
                                 start=(hp == 0), stop=False)
                nc.tensor.matmul(pr_b[:nsz, :], outT_pairs[hp][:, n0:n0 + nsz],
                                 wproj_sb[hp][:, 512:768],
                                 start=(hp == 0), stop=False)
            nc.tensor.matmul(pr_a[:nsz, :], ones_bf[:, 0:nsz],
                             bproj_sb[:, 0:512], start=False, stop=True)
            nc.tensor.matmul(pr_b[:nsz, :], ones_bf[:, 0:nsz],
                             bproj_sb[:, 512:768], start=False, stop=True)
            o_sb = p_out.tile([128, C], FP32, tag="osb", bufs=2)
            nc.scalar.copy(o_sb[:nsz, 0:512], pr_a[:nsz, :])
            nc.scalar.copy(o_sb[:nsz, 512:768], pr_b[:nsz, :])
            nc.sync.dma_start(io["out"][b, n0:n0 + nsz, :], o_sb[:nsz, :])


_PROG = None


def _build_program():
    global _PROG
    if _PROG is not None:
        return _PROG
    nc = bacc.Bacc("TRN2", target_bir_lowering=False, debug=False)
    io = {
        "xT_f32": nc.dram_tensor("xT_f32", [NB, C, N], FP32,
                                 kind="ExternalInput").ap(),
        "xT_bf16": nc.dram_tensor("xT_bf16", [NB, C, N], BF16,
                                  kind="ExternalInput").ap(),
        "wqk": nc.dram_tensor("wqk", [C, 2 * C], FP32,
                              kind="ExternalInput").ap(),
        "wv": nc.dram_tensor("wv", [C, C], BF16, kind="ExternalInput").ap(),
        "wproj": nc.dram_tensor("wproj", [C, C], BF16,
                                kind="ExternalInput").ap(),
        "bproj": nc.dram_tensor("bproj", [1, C], BF16,
                                kind="ExternalInput").ap(),
        "out": nc.dram_tensor("out", [NB, N, C], FP32,
                              kind="ExternalOutput").ap(),
    }
    with tile.TileContext(nc) as tc:
        with ExitStack() as stack:
            tc._ctx = stack
            _emit(tc, io)
    nc.compile()
    _PROG = (nc, io)
    return _PROG


def make_in_maps(x, Wqkv, Wproj, bproj):
    """Shard full inputs into per-core input maps."""
    bf16 = ml_dtypes.bfloat16
    x = np.ascontiguousarray(x, np.float32)
    Wqkv = np.asarray(Wqkv, np.float32)
    wqk = np.ascontiguousarray(Wqkv[:, :2 * C])
    wv = np.ascontiguousarray(Wqkv[:, 2 * C:]).astype(bf16)
    wproj = np.ascontiguousarray(Wproj, np.float32).astype(bf16)
    bp = np.asarray(bproj, np.float32).reshape(1, C).astype(bf16)
    in_maps = []
    for core in range(NCORES):
        xs = x[core * NB:(core + 1) * NB]  # [2, N, C]
        xT = np.ascontiguousarray(xs.transpose(0, 2, 1))  # [2, C, N]
        in_maps.append({
            "xT_f32": xT,
            "xT_bf16": xT.astype(bf16),
            "wqk": wqk,
            "wv": wv,
            "wproj": wproj,
            "bproj": bp,
        })
    return in_maps


def kernel(x, Wqkv, Wproj, bproj):
    nc, _ = _build_program()
    in_maps = make_in_maps(x, Wqkv, Wproj, bproj)
    res = run_bass_kernel_spmd(nc, in_maps, list(range(NCORES)))
    outs = [r["out"] for r in res.results]
    return np.concatenate(outs, axis=0).astype(np.float32)


if __name__ == "__main__":
    _build_program()
    print("BUILD OK")


# revision 38
# speedup vs baseline: 1.0641x; 1.0641x over previous
"""MiTA sparse attention kernel for Trainium2 (8 NeuronCores, Bass/Tile).

Sharding: data-parallel over batch B=16 -> 2 batches per core; all 12 heads
of a batch are processed on the same core.

Math (per batch b, head h; d=64, M=25 experts, kv_topk=12, router_topk=2):
  qkv = x @ Wqkv ; router = AdaptiveAvgPool(q-grid)
  rak = router k^T ; kidx = top12(rak) ; gate = q router^T ; top2 experts/query
  single softmax over {agent logits (25)} U {selected experts' top12 keys}
  out = (e_a @ (softmax(rak*s) @ v) + e_m @ v[kidx]) / denom ; proj.

Implementation notes:
  - selection chain (qk^T, rak, gate, router) in fp32: lower precision flips
    top-k selections (tf32 measured 2.7e-1 rel err; bf16 2.8e-1).
  - value path bf16 (rel-max error ~4e-3 with zero flips).
  - router = pool(q) = pool(x) @ Wq: pooling commutes with the linear map, so
    pool x^T with 2-stage DVE window reduces (independent of the heavy qk^T
    matmuls) and get router^T via a small exact fp32 matmul.
  - moba branch: full-577-key attention weighted by the multiplicity mask
    W[n,j] = sum_m sel[n,m]*mask12[m,j] in {0,1,2} (exact 0/1 matmul in bf16)
    in transposed space so every contraction is matmul-native.
  - softmax runs unstabilized (logit scale ~0.3) = max-subtracted reference.
  - denominators come from ones-augmented value matrices; the divide runs on
    the Pool engine (gpsimd) with a partition broadcast, no DVE involvement.
  - phase order: k-tiles -> v -> rak/top-12 -> PT/av -> q-tiles -> gate/sel
    -> e_a^T -> EW loop -> proj, so the top-k/PT/av chains (DVE/ACT) overlap
    the PE-bound fp32 qk^T matmul window.
  - element-wise work is spread over DVE / ACT / Pool(gpsimd) for occupancy;
    heads' outputs are paired [128, N] so the projection contracts 128 rows.
"""

import sys

for _p in ("/opt/trn_rl_repo",):
    if _p not in sys.path:
        sys.path.insert(0, _p)

from contextlib import ExitStack

import numpy as np
import ml_dtypes

import concourse.bacc as bacc
import concourse.tile as tile
import concourse.mybir as mybir
from concourse.bass_utils import run_bass_kernel_spmd
from concourse.masks import make_identity

FP32 = mybir.dt.float32
BF16 = mybir.dt.bfloat16
ALU = mybir.AluOpType
ACTF = mybir.ActivationFunctionType
AX = mybir.AxisListType

B, N, C = 16, 577, 768
H, D, M, POOL = 12, 64, 25, 5
NB = 2  # batches per core
NCORES = 8
SCALE = float(D) ** -0.5  # 0.125
NEGBIG = -1e30
NTS = [(i * 128, min(128, N - i * 128)) for i in range((N + 127) // 128)]  # 5
CTS = 6  # 128-col tiles per 768
import os
PHASES = int(os.environ.get("MITA_PHASES", "9"))

# adaptive-pool 1D bins of the 24-token grid axis: (start, len)
_BINS = [(int(np.floor(i * 24 / POOL)),
          int(np.ceil((i + 1) * 24 / POOL)) - int(np.floor(i * 24 / POOL)))
         for i in range(POOL)]
# weight 1/(ny*nx) for region m = r*5 + c
_WPOOL = [1.0 / (_BINS[m // POOL][1] * _BINS[m % POOL][1]) for m in range(M)]


def _emit(tc, io):
    nc = tc.nc
    ctx = tc._ctx

    p_const = ctx.enter_context(tc.tile_pool(name="const", bufs=1))
    p_w = ctx.enter_context(tc.tile_pool(name="work", bufs=1))
    p_ew = ctx.enter_context(tc.tile_pool(name="ew", bufs=8))
    p_out = ctx.enter_context(tc.tile_pool(name="pout", bufs=1))
    # PSUM pools: single-bank tiles; 8 banks total (3 + 3 + 2).
    ps_a = ctx.enter_context(tc.tile_pool(name="ps_a", bufs=3, space="PSUM"))
    ps_w = ctx.enter_context(tc.tile_pool(name="ps_w", bufs=3, space="PSUM"))
    ps_v = ctx.enter_context(tc.tile_pool(name="ps_v", bufs=2, space="PSUM"))

    # ---- constants / weights ----
    ident_bf = p_const.tile([128, 128], BF16, tag="idbf")
    make_identity(nc, ident_bf[:])
    ones_bf = p_const.tile([1, 128], BF16, tag="ones")
    nc.vector.memset(ones_bf[:], 1.0)
    wpool = p_const.tile([128, M], FP32, tag="wpool")
    for m in range(M):
        nc.vector.memset(wpool[:, m:m + 1], _WPOOL[m])

    # weight DMAs are interleaved with the first batch's x loads below so
    # the first qk^T matmul isn't stuck behind ~7MB of weight traffic
    wqk_sb, wv_sb, wproj_sb = [], [], []
    bproj_sb = None

    for b in range(NB):
        # ---- load x^T (fp32 + bf16), interleaved with weights on b=0 ----
        xT32 = []
        for kc in range(CTS):
            if b == 0:
                w = p_const.tile([128, 2 * C], FP32, tag=f"wqk{kc}",
                                 name=f"wqk{kc}")
                nc.sync.dma_start(w[:], io["wqk"][kc * 128:(kc + 1) * 128, :])
                wqk_sb.append(w)
            t = p_w.tile([128, N], FP32, tag=f"w{kc}", name=f"xT32_{kc}")
            nc.sync.dma_start(t[:], io["xT_f32"][b, kc * 128:(kc + 1) * 128, :])
            xT32.append(t)
        xTbf = []
        for kc in range(CTS):
            t = p_w.tile([128, N], BF16, tag=f"t{kc}", name=f"xTbf_{kc}")
            nc.sync.dma_start(t[:], io["xT_bf16"][b, kc * 128:(kc + 1) * 128, :])
            xTbf.append(t)
        if b == 0:
            for kc in range(CTS):
                w = p_const.tile([128, C], BF16, tag=f"wv{kc}", name=f"wv{kc}")
                nc.sync.dma_start(w[:], io["wv"][kc * 128:(kc + 1) * 128, :])
                wv_sb.append(w)
            for hp in range(H // 2):
                w = p_const.tile([128, C], BF16, tag=f"wp{hp}", name=f"wp{hp}")
                nc.sync.dma_start(w[:], io["wproj"][hp * 128:(hp + 1) * 128, :])
                wproj_sb.append(w)
            bproj_sb = p_const.tile([1, C], BF16, tag="bproj")
            nc.sync.dma_start(bproj_sb[:], io["bproj"][:, :])

        # ---- xpool: 2-stage windowed sums over the 24x24 token grid ----
        xpool = []
        for kc in range(CTS):
            grid = xT32[kc][:, 0:576].rearrange("p (y x) -> p y x", x=24)
            tmp = p_w.tile([128, POOL * 24], FP32, tag="xptmp", bufs=2)
            tv = tmp[:].rearrange("p (w y) -> p w y", y=24)
            for wi, (x0, nx) in enumerate(_BINS):
                nc.vector.tensor_reduce(tv[:, wi, :].unsqueeze(-1),
                                        grid[:, :, x0:x0 + nx],
                                        axis=AX.X, op=ALU.add)
            xp = p_w.tile([128, 32], FP32, tag=f"xp{kc}", name=f"xp{kc}")
            for vi, (y0, ny) in enumerate(_BINS):
                nc.vector.tensor_reduce(
                    xp[:, vi * POOL:(vi + 1) * POOL]
                    .rearrange("p (w o) -> p w o", o=1),
                    tv[:, :, y0:y0 + ny], axis=AX.X, op=ALU.add)
            nc.gpsimd.tensor_mul(xp[:, 0:M], xp[:, 0:M], wpool[:])
            nc.gpsimd.memset(xp[:, M:32], 0.0)
            xpool.append(xp)

        # ---- router^T via exact fp32 matmul: [128c, 32] per ct ----
        routerT32, routerTbf = [], []
        for ct in range(CTS):
            pr = ps_v.tile([128, 32], FP32, tag="bank_v")
            for kc in range(CTS):
                nc.tensor.matmul(pr[:, :], wqk_sb[kc][:, ct * 128:(ct + 1) * 128],
                                 xpool[kc][:, :], start=(kc == 0), stop=(kc == 5))
            t32 = p_w.tile([128, 32], FP32, tag=f"rt{ct}", name=f"rT32_{ct}")
            nc.scalar.copy(t32[:], pr[:])
            tbf = p_w.tile([128, 32], BF16, tag=f"rtb{ct}", name=f"rTbf_{ct}")
            nc.vector.tensor_copy(tbf[:], t32[:])
            routerT32.append(t32)
            routerTbf.append(tbf)

        # ---- qk^T fp32 (+bf16 copy): k tiles first so the rak/top-12
        # chain overlaps the remaining PE-bound matmuls ----
        qkT32, qkTbf = [None] * (2 * CTS), [None] * (2 * CTS)
        for ct in list(range(CTS, 2 * CTS)) + list(range(CTS)):
            pa = ps_a.tile([128, 512], FP32, tag="bank_a")
            pb = ps_a.tile([128, 65], FP32, tag="bank_a")
            for kc in range(CTS):
                nc.tensor.matmul(pa[:, :], wqk_sb[kc][:, ct * 128:(ct + 1) * 128],
                                 xT32[kc][:, 0:512], start=(kc == 0), stop=(kc == 5))
            for kc in range(CTS):
                nc.tensor.matmul(pb[:, :], wqk_sb[kc][:, ct * 128:(ct + 1) * 128],
                                 xT32[kc][:, 512:577], start=(kc == 0), stop=(kc == 5))
            tag32 = f"q{ct}" if ct < CTS else f"k{ct - CTS}"
            t32 = p_w.tile([128, N], FP32, tag=tag32, name=f"qkT32_{ct}")
            nc.scalar.copy(t32[:, 0:512], pa[:])
            nc.scalar.copy(t32[:, 512:577], pb[:])
            tbf = p_w.tile([128, N], BF16, tag=f"qkbf{ct}", name=f"qkTbf_{ct}")
            if ct < CTS:
                # q copies land in the congested gate window: use ACT there
                nc.scalar.copy(tbf[:, :], t32[:, :])
            else:
                nc.gpsimd.tensor_copy(tbf[:, :], t32[:, :])
            qkT32[ct] = t32
            qkTbf[ct] = tbf

        # ---- v natural bf16 with ones-augmentation: [n, 12*65] ----
        v_sb = []
        for i, (n0, nsz) in enumerate(NTS):
            pa = ps_a.tile([128, 512], FP32, tag="bank_a")
            pb = ps_a.tile([128, 256], FP32, tag="bank_a")
            for kc in range(CTS):
                nc.tensor.matmul(pa[:nsz, :], xTbf[kc][:, n0:n0 + nsz],
                                 wv_sb[kc][:, 0:512], start=(kc == 0), stop=(kc == 5))
            for kc in range(CTS):
                nc.tensor.matmul(pb[:nsz, :], xTbf[kc][:, n0:n0 + nsz],
                                 wv_sb[kc][:, 512:768], start=(kc == 0), stop=(kc == 5))
            t = p_w.tile([128, H * 65], BF16, tag=f"v{i}", name=f"v_{i}")
            nc.scalar.copy(
                t[:nsz].rearrange("p (h e) -> p h e", e=65)[:, 0:8, 0:64],
                pa[:nsz].rearrange("p (h e) -> p h e", e=64))
            nc.scalar.copy(
                t[:nsz].rearrange("p (h e) -> p h e", e=65)[:, 8:12, 0:64],
                pb[:nsz].rearrange("p (h e) -> p h e", e=64))
            nc.gpsimd.memset(
                t[:nsz].rearrange("p (h e) -> p h e", e=65)[:, :, 64:65], 1.0)
            v_sb.append(t)

        if PHASES < 2:
            continue
        # ---- rak (fp32) -> top12 threshold -> mask12, early (needs only
        # k tiles + router) ----
        mask12_g = []
        for g2 in range(6):
            rak_sb = p_w.tile([64, N], FP32, tag="rak_sb", name="rak_sb",
                              bufs=2)
            for hh in range(2):
                h = g2 * 2 + hh
                b32 = hh * 32
                rk = (h % 2) * 64
                kt = 6 + h // 2
                ra = ps_a.tile([32, 512], FP32, tag="bank_a", name="ra")
                rb_ = ps_a.tile([32, 65], FP32, tag="bank_a", name="rb_")
                nc.tensor.matmul(ra[:, :],
                                 routerT32[h // 2][rk:rk + 64, :],
                                 qkT32[kt][rk:rk + 64, 0:512],
                                 start=True, stop=True)
                nc.tensor.matmul(rb_[:, :],
                                 routerT32[h // 2][rk:rk + 64, :],
                                 qkT32[kt][rk:rk + 64, 512:577],
                                 start=True, stop=True)
                nc.scalar.copy(rak_sb[b32:b32 + 32, 0:512], ra[:, :])
                nc.scalar.copy(rak_sb[b32:b32 + 32, 512:577], rb_[:, :])
            r8 = p_w.tile([64, 8], FP32, tag="r8", bufs=2)
            rr = p_w.tile([64, N], FP32, tag="rr", name="rr", bufs=1)
            r8b = p_w.tile([64, 8], FP32, tag="r8b", bufs=2)
            nc.vector.max(out=r8[:], in_=rak_sb[:])
            nc.vector.match_replace(out=rr[:], in_to_replace=r8[:],
                                    in_values=rak_sb[:], imm_value=NEGBIG)
            nc.vector.max(out=r8b[:], in_=rr[:])
            mask12 = p_w.tile([64, N], BF16, tag=f"mask12_{g2}",
                              name=f"mask12_{g2}")
            nc.gpsimd.tensor_scalar(mask12[:], rak_sb[:], r8b[:, 3:4], None,
                                    op0=ALU.is_ge)
            mask12_g.append(mask12)

        if PHASES < 3:
            continue
        # ---- PT = exp(SCALE * rak^T) per j-chunk, all heads packed ----
        PT_e, PT_o = [], []
        for i, (j0, jsz) in enumerate(NTS):
            rt_e = ps_w.tile([128, 6 * 32], FP32, tag="bank_w")
            rt_o = ps_w.tile([128, 6 * 32], FP32, tag="bank_w")
            for h in range(H):
                rk = (h % 2) * 64
                dst = rt_o if (h % 2) else rt_e
                nc.tensor.matmul(
                    dst[:jsz, (h // 2) * 32:(h // 2 + 1) * 32],
                    qkTbf[6 + h // 2][rk:rk + 64, j0:j0 + jsz],
                    routerTbf[h // 2][rk:rk + 64, :],
                    start=True, stop=True)
            te = p_w.tile([128, 6 * 32], BF16, tag=f"pte{i}", name=f"PTe{i}")
            to = p_w.tile([128, 6 * 32], BF16, tag=f"pto{i}", name=f"PTo{i}")
            nc.scalar.activation(te[:jsz, :], rt_e[:jsz, :], ACTF.Exp,
                                 scale=SCALE)
            nc.scalar.activation(to[:jsz, :], rt_o[:jsz, :], ACTF.Exp,
                                 scale=SCALE)
            PT_e.append(te)
            PT_o.append(to)

        # ---- agent values av = (PT^T v)/colsum, per-head [32, 65] ----
        av_h = []
        for h in range(H):
            PTx = PT_o if (h % 2) else PT_e
            sg = (h // 2) * 32
            t = p_w.tile([32, 65], BF16, tag=f"av_{h}", name=f"av_{h}")
            nc.vector.memset(t[:], 0.0)
            au = ps_w.tile([32, 65], FP32, tag="bank_w", name="au")
            for i, (j0, jsz) in enumerate(NTS):
                nc.tensor.matmul(
                    au[:, :],
                    PTx[i][:jsz, sg:sg + 32],
                    v_sb[i][:jsz, h * 65:(h + 1) * 65],
                    start=(i == 0), stop=(i == 4))
            rp = p_w.tile([32, 1], FP32, tag="avrec", bufs=2)
            nc.vector.reciprocal(rp[0:M, :], au[0:M, 64:65])
            nc.vector.tensor_scalar(t[0:M, 0:64], au[0:M, 0:64],
                                    rp[0:M, :], None, op0=ALU.mult)
            nc.gpsimd.memset(t[0:M, 64:65], 1.0)
            av_h.append(t)

        if PHASES < 4:
            continue
        # ---- gate (fp32, natural) -> sel; transpose sel to [m, n] ----
        selT = [p_w.tile([64, N], BF16, tag=f"selT{g2}", name=f"selT{g2}")
                for g2 in range(6)]
        for i, (n0, nsz) in enumerate(NTS):
            # mms with different lhsT partition bases must not share a psum
            # bank unfenced (HW crash) -> split by head parity
            gp_e = ps_v.tile([128, 6 * 32], FP32, tag="bank_v")
            gp_o = ps_v.tile([128, 6 * 32], FP32, tag="bank_v")
            for h in range(H):
                rk = (h % 2) * 64
                dst = gp_o if (h % 2) else gp_e
                nc.tensor.matmul(
                    dst[:nsz, (h // 2) * 32:(h // 2 + 1) * 32],
                    qkT32[h // 2][rk:rk + 64, n0:n0 + nsz],
                    routerT32[h // 2][rk:rk + 64, :],
                    start=True, stop=True)
            gate_sb = p_w.tile([128, H * 32], FP32, tag="gate", bufs=3)
            gv = gate_sb[:nsz].rearrange("p (h e) -> p h e", e=32)
            nc.scalar.copy(gv[:, 0:H:2, :],
                           gp_e[:nsz].rearrange("p (h e) -> p h e", e=32))
            nc.scalar.copy(gv[:, 1:H:2, :],
                           gp_o[:nsz].rearrange("p (h e) -> p h e", e=32))
            nc.gpsimd.memset(
                gate_sb[:nsz].rearrange("p (h e) -> p h e", e=32)[:, :, M:32],
                NEGBIG)
            sel_sb = p_w.tile([128, H * 32], BF16, tag="sel", bufs=3)
            m8 = p_w.tile([128, 8], FP32, tag="m8", bufs=3)
            for h in range(H):
                seg = slice(h * 32, (h + 1) * 32)
                nc.vector.max(out=m8[:nsz, :], in_=gate_sb[:nsz, seg])
                nc.gpsimd.tensor_scalar(
                    sel_sb[:nsz, seg], gate_sb[:nsz, seg], m8[:nsz, 1:2], None,
                    op0=ALU.is_ge)
            for ch in range(3):
                pt = ps_w.tile([128, 128], BF16, tag="bank_w")
                nc.tensor.matmul(pt[0:128, 0:nsz],
                                 sel_sb[:nsz, ch * 128:(ch + 1) * 128],
                                 ident_bf[0:nsz, 0:nsz],
                                 is_transpose=True, start=True, stop=True,
                                 skip_group_check=True)
                nc.scalar.copy(selT[2 * ch][:, n0:n0 + nsz], pt[0:64, 0:nsz])
                nc.vector.tensor_copy(selT[2 * ch + 1][:, n0:n0 + nsz],
                                      pt[64:128, 0:nsz])

        # ---- e_a^T direct: exp(SCALE * (router^T)^T q^T) per head ----
        eaT = []
        for h in range(H):
            rk = (h % 2) * 64
            ga = ps_w.tile([32, 512], FP32, tag="bank_w", name="ga")
            gb = ps_w.tile([32, 65], FP32, tag="bank_w", name="gb")
            nc.tensor.matmul(ga[:, :], routerTbf[h // 2][rk:rk + 64, 0:32],
                             qkTbf[h // 2][rk:rk + 64, 0:512],
                             start=True, stop=True)
            nc.tensor.matmul(gb[:, :], routerTbf[h // 2][rk:rk + 64, 0:32],
                             qkTbf[h // 2][rk:rk + 64, 512:577],
                             start=True, stop=True)
            tag = f"q{h}" if h < 6 else f"ea{h - 6}"
            t = p_w.tile([32, N], BF16, tag=tag, name=f"eaT{h}")
            nc.scalar.activation(t[0:32, 0:512], ga[:, :], ACTF.Exp, scale=SCALE)
            nc.scalar.activation(t[0:32, 512:577], gb[:, :], ACTF.Exp,
                                 scale=SCALE)
            eaT.append(t)

        # ---- EW loop per head: W^T, qk^T, exp, mask-multiply, value ----
        outT_pairs = [p_out.tile([128, N], BF16, tag=f"outP{hp}",
                                 name=f"outP{hp}") for hp in range(H // 2)]
        for h in range(H):
            g2 = h // 2
            b32 = (h % 2) * 32
            rk = (h % 2) * 64
            mask12 = mask12_g[g2]
            ew = []
            for i, (j0, jsz) in enumerate(NTS):
                # W^T[j, n] = mask12^T @ sel^T  (exact in bf16)
                wt_a = ps_w.tile([128, 512], FP32, tag="bank_w")
                wt_b = ps_w.tile([128, 65], FP32, tag="bank_w")
                nc.tensor.matmul(wt_a[:jsz, :],
                                 mask12[b32:b32 + 32, j0:j0 + jsz],
                                 selT[g2][b32:b32 + 32, 0:512],
                                 start=True, stop=True)
                nc.tensor.matmul(wt_b[:jsz, :],
                                 mask12[b32:b32 + 32, j0:j0 + jsz],
                                 selT[g2][b32:b32 + 32, 512:577],
                                 start=True, stop=True)
                # qk^T[j, n] (bf16 value path)
                qm_a = ps_a.tile([128, 512], FP32, tag="bank_a")
                qm_b = ps_a.tile([128, 65], FP32, tag="bank_a")
                kt = 6 + h // 2
                qt = h // 2
                nc.tensor.matmul(qm_a[:jsz, :],
                                 qkTbf[kt][rk:rk + 64, j0:j0 + jsz],
                                 qkTbf[qt][rk:rk + 64, 0:512],
                                 start=True, stop=True)
                nc.tensor.matmul(qm_b[:jsz, :],
                                 qkTbf[kt][rk:rk + 64, j0:j0 + jsz],
                                 qkTbf[qt][rk:rk + 64, 512:577],
                                 start=True, stop=True)
                e_a = p_w.tile([128, 512], BF16, tag="exp_a", bufs=6)
                e_b = p_w.tile([128, 65], BF16, tag="exp_b", bufs=6)
                nc.scalar.activation(e_a[:jsz, :], qm_a[:jsz, :], ACTF.Exp,
                                     scale=SCALE)
                nc.scalar.activation(e_b[:jsz, :], qm_b[:jsz, :], ACTF.Exp,
                                     scale=SCALE)
                t = p_ew.tile([128, N], BF16, tag="ew")
                nc.vector.tensor_tensor(t[:jsz, 0:512], e_a[:jsz, :],
                                        wt_a[:jsz, :], op=ALU.mult)
                nc.vector.tensor_tensor(t[:jsz, 512:577], e_b[:jsz, :],
                                        wt_b[:jsz, :], op=ALU.mult)
                ew.append(t)
            # numT [65, 577] = av_aug^T e_a^T + v_aug^T EW^T
            val_a = ps_v.tile([65, 512], FP32, tag="bank_v")
            val_b = ps_v.tile([65, 65], FP32, tag="bank_v")
            nc.tensor.matmul(val_a[:, :], av_h[h][0:32, :],
                             eaT[h][0:32, 0:512],
                             start=True, stop=False)
            nc.tensor.matmul(val_b[:, :], av_h[h][0:32, :],
                             eaT[h][0:32, 512:577],
                             start=True, stop=False)
            for i, (j0, jsz) in enumerate(NTS):
                nc.tensor.matmul(val_a[:, :],
                                 v_sb[i][:jsz, h * 65:(h + 1) * 65],
                                 ew[i][:jsz, 0:512],
                                 start=False, stop=(i == 4))
                nc.tensor.matmul(val_b[:, :],
                                 v_sb[i][:jsz, h * 65:(h + 1) * 65],
                                 ew[i][:jsz, 512:577],
                                 start=False, stop=(i == 4))
            numT = p_w.tile([65, N], FP32, tag="numT", name="numT", bufs=2)
            nc.scalar.copy(numT[:, 0:512], val_a[:])
            nc.scalar.copy(numT[:, 512:577], val_b[:])
            den1 = p_w.tile([1, N], FP32, tag="den1", bufs=1)
            nc.sync.dma_start(den1[0:1, :], numT[64:65, :])
            nc.vector.reciprocal(den1[0:1, :], den1[0:1, :])
            # broadcast the reciprocal and multiply on the Pool engine
            rb = p_w.tile([64, N], FP32, tag="rb", name="rb", bufs=1)
            nc.gpsimd.partition_broadcast(rb[:, :], den1[0:1, :], channels=64)
            dst = outT_pairs[h // 2]
            rows = slice((h % 2) * 64, (h % 2) * 64 + 64)
            nc.gpsimd.tensor_tensor(dst[rows, :], numT[0:64, :], rb[:, :],
                                    op=ALU.mult)

        # ---- proj: out[n, c'] = attnout @ Wproj + bproj, head-paired ----
        if PHASES < 9:
            continue
        for i, (n0, nsz) in enumerate(NTS):
            pr_a = ps_w.tile([128, 512], FP32, tag="bank_w")
            pr_b = ps_w.tile([128, 256], FP32, tag="bank_w")
            for hp in range(H // 2):
                nc.tensor.matmul(pr_a[:nsz, :], outT_pairs[hp][:, n0:n0 + nsz],
                                 wproj_sb[hp][:, 0:512],
                                 start=(hp == 0), stop=False)
                nc.tensor.matmul(pr_b[:nsz, :], outT_pairs[hp][:, n0:n0 + nsz],
                                 wproj_sb[hp][:, 512:768],
                                 start=(hp == 0), stop=False)
            nc.tensor.matmul(pr_a[:nsz, :], ones_bf[:, 0:nsz],
                             bproj_sb[:, 0:512], start=False, stop=True)
            nc.tensor.matmul(pr_b[:nsz, :], ones_bf[:, 0:nsz],
                             bproj_sb[:, 512:768], start=False, stop=True)
            o_sb = p_out.tile([128, C], FP32, tag="osb", bufs=2)
            nc.scalar.copy(o_sb[:nsz, 0:512], pr_a[:nsz, :])
            nc.scalar.copy(o_sb[:nsz, 512:768], pr_b[:nsz, :])
            nc.sync.dma_start(io["out"][b, n0:n0 + nsz, :], o_sb[:nsz, :])


_PROG = None


def _build_program():
    global _PROG
    if _PROG is not None:
        return _PROG
    nc = bacc.Bacc("TRN2", target_bir_lowering=False, debug=False)
    io = {
        "xT_f32": nc.dram_tensor("xT_f32", [NB, C, N], FP32,
                                 kind="ExternalInput").ap(),
        "xT_bf16": nc.dram_tensor("xT_bf16", [NB, C, N], BF16,
                                  kind="ExternalInput").ap(),
        "wqk": nc.dram_tensor("wqk", [C, 2 * C], FP32,
                              kind="ExternalInput").ap(),
        "wv": nc.dram_tensor("wv", [C, C], BF16, kind="ExternalInput").ap(),
        "wproj": nc.dram_tensor("wproj", [C, C], BF16,
                                kind="ExternalInput").ap(),
        "bproj": nc.dram_tensor("bproj", [1, C], BF16,
                                kind="ExternalInput").ap(),
        "out": nc.dram_tensor("out", [NB, N, C], FP32,
                              kind="ExternalOutput").ap(),
    }
    with tile.TileContext(nc) as tc:
        with ExitStack() as stack:
            tc._ctx = stack
            _emit(tc, io)
    nc.compile()
    _PROG = (nc, io)
    return _PROG


def make_in_maps(x, Wqkv, Wproj, bproj):
    """Shard full inputs into per-core input maps."""
    bf16 = ml_dtypes.bfloat16
    x = np.ascontiguousarray(x, np.float32)
    Wqkv = np.asarray(Wqkv, np.float32)
    wqk = np.ascontiguousarray(Wqkv[:, :2 * C])
    wv = np.ascontiguousarray(Wqkv[:, 2 * C:]).astype(bf16)
    wproj = np.ascontiguousarray(Wproj, np.float32).astype(bf16)
    bp = np.asarray(bproj, np.float32).reshape(1, C).astype(bf16)
    in_maps = []
    for core in range(NCORES):
        xs = x[core * NB:(core + 1) * NB]  # [2, N, C]
        xT = np.ascontiguousarray(xs.transpose(0, 2, 1))  # [2, C, N]
        in_maps.append({
            "xT_f32": xT,
            "xT_bf16": xT.astype(bf16),
            "wqk": wqk,
            "wv": wv,
            "wproj": wproj,
            "bproj": bp,
        })
    return in_maps


def kernel(x, Wqkv, Wproj, bproj):
    nc, _ = _build_program()
    in_maps = make_in_maps(x, Wqkv, Wproj, bproj)
    res = run_bass_kernel_spmd(nc, in_maps, list(range(NCORES)))
    outs = [r["out"] for r in res.results]
    return np.concatenate(outs, axis=0).astype(np.float32)


if __name__ == "__main__":
    _build_program()
    print("BUILD OK")
